# revision 16
# baseline (speedup 1.0000x reference)
"""Cross-attention block (q from z_hsi, k/v from z_msi, softmax over 6400
pixels, residual + gamma) on 8 Trainium2 NeuronCores.

Sharding: the (batch=2, N=6400) query-pixel space is split into 8 shards of
1600 pixels (4 shards per batch element). Each core computes its shard's
attention output against the full key/value set of its batch element; the
host slices inputs and concatenates outputs (no device collectives).

Math restructuring vs the naive form:
  * softmax over j is invariant to adding any per-i constant, so the K bias
    (bk) is dropped entirely, and
      E[j,i] = K[:,j]^T Q[:,i]  ==  zm[:,j]^T (Wk^T Wq zq + Wk^T bq)[:,i]
    so K and Q are never materialized: one [64 x 1600] "QK" projection
    (host precomputes Wq^T Wk and Wk^T bq) replaces both.
  * The V bias folds out of the attention matmul:  (V+bv) P = V P + bv * d,
    so after normalizing by d it becomes "+ gamma*bv" on the residual.
  * E is computed in float32r (TF32-like, full PE rate, ~1e-4); the
    post-exp side (P, V) uses bf16, whose quantization noise averages out
    over the 6400-deep attention sum.
  * d[i] = sum_j exp-sums are PE ones-matmuls over DVE pair/triple-summed
    exp tiles (halves/thirds the PE cost of the denominator).
  * exp runs on ACT straight out of PSUM in [128, 3x400] strided batches
    (68 instructions instead of 200) while d/PV matmuls trail one group
    behind (software pipeline, PE never waits on the exp it just enabled).
"""
import sys

sys.path.insert(0, "/opt/trn_rl_repo")

import ml_dtypes
import numpy as np
import concourse.bass as bass  # noqa: F401
import concourse.tile as tile
from concourse import bacc, mybir
from concourse.bass_utils import run_bass_kernel_spmd

B, CH, CM, CO = 2, 128, 64, 128
H = W = 80
N = H * W                # 6400 key/value pixels per batch element
NCORES = 8
NI = (B * N) // NCORES   # 1600 query pixels per core
JT = N // 128            # 50 key tiles
F32 = mybir.dt.float32
F32R = mybir.dt.float32r
BF16 = mybir.dt.bfloat16

IBS = 400
I_BLOCKS = [(k * IBS, IBS) for k in range(NI // IBS)]
# jt groups of 2: one exp instruction per group
GROUPS = [tuple(range(g, min(g + 2, JT))) for g in range(0, JT, 2)]


def _build(repeat=1):
    """repeat>1 wraps the whole per-core compute in an on-device For_i loop;
    used only by the perf harness to measure HW time via wall-clock slope."""
    nc = bacc.Bacc(None, target_bir_lowering=False)
    zq = nc.declare_dram_parameter("zq", [CH, NI], F32R, isOutput=False)
    zm = nc.declare_dram_parameter("zm", [128, N], F32R, isOutput=False)
    wqk = nc.declare_dram_parameter("wqk", [CH, 128], F32R, isOutput=False)
    bkq = nc.declare_dram_parameter("bkq", [128, 1], F32, isOutput=False)
    wvT = nc.declare_dram_parameter("wvT", [128, CO], F32R, isOutput=False)
    gbv = nc.declare_dram_parameter("gbv", [CO, 1], F32, isOutput=False)
    gsc = nc.declare_dram_parameter("gsc", [1, 1], F32, isOutput=False)
    ones = nc.declare_dram_parameter("ones", [128, 1], F32R, isOutput=False)
    out = nc.declare_dram_parameter("out", [CO, NI], F32, isOutput=True)

    with tile.TileContext(nc) as tc:
        with (
            tc.tile_pool(name="big", bufs=1) as big,
            tc.tile_pool(name="expp", bufs=4) as expp,
            tc.tile_pool(name="work", bufs=2) as work,
            tc.tile_pool(name="pse", bufs=2, space="PSUM") as pse,
            tc.tile_pool(name="pspv", bufs=2, space="PSUM") as pspv,
        ):
            zm_sb = big.tile([128, N], F32R)
            nc.sync.dma_start(zm_sb[:], zm[:])
            zq_sb = big.tile([CH, NI], F32R)
            nc.sync.dma_start(zq_sb[:], zq[:])
            wqk_sb = big.tile([CH, 128], F32R)
            nc.sync.dma_start(wqk_sb[:], wqk[:])
            bkq_sb = big.tile([128, 1], F32)
            nc.sync.dma_start(bkq_sb[:], bkq[:])
            wv_sb = big.tile([128, CO], F32R)
            nc.sync.dma_start(wv_sb[:], wvT[:])
            gbv_sb = big.tile([CO, 1], F32)
            nc.sync.dma_start(gbv_sb[:], gbv[:])
            gsc_sb = big.tile([1, 1], F32)
            nc.sync.dma_start(gsc_sb[:], gsc[:])
            ones_sb = big.tile([128, 1], F32R)
            nc.sync.dma_start(ones_sb[:], ones[:])

            from contextlib import nullcontext
            rep_ctx = tc.For_i(0, repeat, 1) if repeat > 1 else nullcontext()
            with rep_ctx:
                _emit_body(nc, tc, big, expp, work, pse, pspv,
                           zm_sb, zq_sb, wqk_sb, bkq_sb, wv_sb,
                           gbv_sb, gsc_sb, ones_sb, out)

    nc.finalize()
    return nc


def _emit_body(nc, tc, big, expp, work, pse, pspv,
               zm_sb, zq_sb, wqk_sb, bkq_sb, wv_sb,
               gbv_sb, gsc_sb, ones_sb, out):
    # residual (+ folded gamma*bv), exact fp32 bits of z_hsi
    zqp = big.tile([CH, NI], F32)
    nc.vector.tensor_scalar_add(zqp[:], zq_sb[:].bitcast(F32), gbv_sb[:])

    # QK[c, i] = (Wk^T Wq zq + Wk^T bq)[c, i]   -> E = zm^T QK
    QK_sb = big.tile([128, NI], F32R)
    for c0 in range(0, NI, 512):
        cs = min(512, NI - c0)
        pq = pse.tile([128, 1024], F32, tag="e")
        nc.tensor.matmul(pq[:, :cs], wqk_sb[:], zq_sb[:, c0:c0 + cs],
                         start=True, stop=True)
        nc.vector.tensor_scalar_add(QK_sb[:, c0:c0 + cs], pq[:, :cs],
                                    bkq_sb[:])

    # VT tiles: VT[j, o] = sum_c zm[c, j] Wv[o, c]
    VT_sb = big.tile([128, JT * CO], F32R)
    for g0 in range(0, JT, 4):
        nq = min(4, JT - g0)
        pvt = pse.tile([128, 1024], F32, tag="e")
        for jj in range(nq):
            j0 = (g0 + jj) * 128
            nc.tensor.matmul(pvt[:, jj * 128:(jj + 1) * 128],
                             zm_sb[:, j0:j0 + 128], wv_sb[:],
                             start=True, stop=True)
        nc.vector.tensor_copy(VT_sb[:, g0 * 128:(g0 + nq) * 128],
                              pvt[:, :nq * 128])

    # main attention loop; d/PV matmuls trail one exp-group behind
    for i0, ibs in I_BLOCKS:
        pv = pspv.tile([128, 512], F32, tag="pv")
        dsum = pspv.tile([128, 512], F32, tag="d")

        def emit_dpv(p3_prev, s3_prev, gi):
            grp = GROUPS[gi]
            nc.tensor.matmul(
                dsum[:1, :ibs], ones_sb[:], s3_prev[:, :ibs],
                start=(gi == 0), stop=(gi == len(GROUPS) - 1),
                skip_group_check=True)
            for t, jt in enumerate(grp):
                nc.tensor.matmul(
                    pv[:, :ibs],
                    VT_sb[:, jt * 128:(jt + 1) * 128],
                    p3_prev[:, t * 512:t * 512 + ibs],
                    start=(jt == 0), stop=(jt == JT - 1),
                    skip_group_check=True)

        from collections import deque
        pending = deque()
        for gi, grp in enumerate(GROUPS):
            m = len(grp)
            e3 = pse.tile([128, 1024], F32, tag="e")
            for t, jt in enumerate(grp):
                nc.tensor.matmul(
                    e3[:, t * 512:t * 512 + ibs],
                    zm_sb[:, jt * 128:(jt + 1) * 128],
                    QK_sb[:, i0:i0 + ibs],
                    start=True, stop=True)
            p3 = expp.tile([128, 1024], F32R, tag="p")
            e3v = e3[:].rearrange("p (t x) -> p t x", x=512)[:, :m, :ibs]
            p3v = p3[:].rearrange("p (t x) -> p t x", x=512)[:, :m, :ibs]
            nc.scalar.activation(p3v, e3v, mybir.ActivationFunctionType.Exp)
            s3 = expp.tile([128, 512], F32R, tag="s")
            nc.vector.tensor_add(s3[:, :ibs], p3[:, 0:ibs],
                                 p3[:, 512:512 + ibs])
            pending.append((p3, s3, gi))
            if len(pending) > 2:
                emit_dpv(*pending.popleft())
        while pending:
            emit_dpv(*pending.popleft())

        # normalize: out = PV * (gamma/d) + zqp
        d_inv = work.tile([1, 512], F32, tag="dinv")
        nc.vector.reciprocal(d_inv[:, :ibs], dsum[:1, :ibs])
        d_g = work.tile([1, 512], F32, tag="dg")
        nc.vector.tensor_scalar_mul(d_g[:, :ibs], d_inv[:, :ibs], gsc_sb[:])
        b_sb = work.tile([128, 512], F32, tag="bsb")
        nc.gpsimd.partition_broadcast(b_sb[:, :ibs], d_g[:1, :ibs])
        t_sb = work.tile([128, 512], F32, tag="tsb")
        nc.vector.tensor_mul(t_sb[:, :ibs], pv[:, :ibs], b_sb[:, :ibs])
        o_sb = work.tile([128, 512], F32, tag="osb")
        nc.vector.tensor_add(o_sb[:, :ibs], t_sb[:, :ibs],
                             zqp[:, i0:i0 + ibs])
        nc.sync.dma_start(out[:, i0:i0 + ibs], o_sb[:, :ibs])


_cached_nc = None


def kernel(z_hsi, z_msi, Wq, bq, Wk, bk, Wv, bv, gamma):
    global _cached_nc
    if _cached_nc is None:
        _cached_nc = _build()
    nc = _cached_nc

    z_hsi = np.asarray(z_hsi, dtype=np.float32).reshape(B, CH, N)
    z_msi = np.ascontiguousarray(np.asarray(z_msi, dtype=np.float32).reshape(B, CM, N))
    Wq64 = np.asarray(Wq, dtype=np.float64)
    Wk64 = np.asarray(Wk, dtype=np.float64)
    bq64 = np.asarray(bq, dtype=np.float64)
    # QK folding: E = zm^T (Wk^T Wq zq + Wk^T bq); bk cancels in softmax.
    # All CM=64 contractions are zero-padded to 128: K=64 matmuls run ~2x
    # slower per column on TRN2 than K=128.
    wqk_h = np.zeros((CH, 128), np.float32)
    wqk_h[:, :CM] = (Wq64.T @ Wk64).astype(np.float32)
    bkq_h = np.zeros((128, 1), np.float32)
    bkq_h[:CM, 0] = (Wk64.T @ bq64).astype(np.float32)
    wvT_h = np.zeros((128, CO), np.float32)
    wvT_h[:CM] = np.asarray(Wv, np.float32).T
    z_msi_pad = np.zeros((B, 128, N), np.float32)
    z_msi_pad[:, :CM] = z_msi
    g = float(np.asarray(gamma, dtype=np.float32).reshape(-1)[0])
    gbv = np.ascontiguousarray((g * np.asarray(bv, np.float32)).reshape(CO, 1))
    gsc = np.full((1, 1), g, dtype=np.float32)
    ones = np.ones((128, 1), dtype=np.float32)

    shards_per_b = NCORES // B
    in_maps = []
    for c in range(NCORES):
        b, s = c // shards_per_b, (c % shards_per_b) * NI
        in_maps.append({
            "zq": np.ascontiguousarray(z_hsi[b][:, s:s + NI]),
            "zm": z_msi_pad[b],
            "wqk": wqk_h, "bkq": bkq_h, "wvT": wvT_h,
            "gbv": gbv, "gsc": gsc, "ones": ones,
        })

    res = run_bass_kernel_spmd(nc, in_maps, core_ids=list(range(NCORES)))

    out = np.empty((B, CH, N), dtype=np.float32)
    for c in range(NCORES):
        b, s = c // shards_per_b, (c % shards_per_b) * NI
        out[b][:, s:s + NI] = res.results[c]["out"]
    return out.reshape(B, CH, H, W)


# revision 22
# speedup vs baseline: 4323.5870x; 4323.5870x over previous
"""Cross-attention block (q from z_hsi, k/v from z_msi, softmax over 6400
pixels, residual + gamma) on 8 Trainium2 NeuronCores.

Sharding: the (batch=2, N=6400) query-pixel space is split into 8 shards of
1600 pixels (4 shards per batch element). Each core computes its shard's
attention output against the full key/value set of its batch element; the
host slices inputs and concatenates outputs (no device collectives).

Math restructuring vs the naive form:
  * softmax over j is invariant to adding any per-i constant, so the K bias
    (bk) is dropped entirely, and
      E[j,i] = K[:,j]^T Q[:,i]  ==  zm[:,j]^T (Wk^T Wq zq + Wk^T bq)[:,i]
    so K and Q are never materialized: one [64 x 1600] "QK" projection
    (host precomputes Wq^T Wk and Wk^T bq) replaces both.
  * The V bias folds out of the attention matmul:  (V+bv) P = V P + bv * d,
    so after normalizing by d it becomes "+ gamma*bv" on the residual.
  * Everything on the PE runs in float32r (TF32-like precision at full PE
    rate; measured rms error ~1.5e-4 per matmul vs ~2e-3 for bf16, and
    measured FASTER than bf16 on this silicon: 245 vs 276 ns/matmul).
  * All K=64 contractions (z_msi channels) are zero-padded to K=128 --
    K=64 matmuls measure ~2x slower per column than K=128.
  * d[i] = sum_j exp: DVE pair-sums adjacent exp tiles, then one PE
    ones-matmul per pair (halves the PE denominator cost).
  * exp runs on ACT straight out of PSUM in [128, 2x400] strided batches;
    d/PV matmuls trail two exp-groups behind (software pipeline, the PE
    never waits on the exp it just enabled).  The gamma/d broadcast runs
    on the otherwise-idle GPSIMD.
"""
import sys

sys.path.insert(0, "/opt/trn_rl_repo")

import ml_dtypes
import numpy as np
import concourse.bass as bass  # noqa: F401
import concourse.tile as tile
from concourse import bacc, mybir
from concourse.bass_utils import run_bass_kernel_spmd

B, CH, CM, CO = 2, 128, 64, 128
H = W = 80
N = H * W                # 6400 key/value pixels per batch element
NCORES = 8
NI = (B * N) // NCORES   # 1600 query pixels per core
JT = N // 128            # 50 key tiles
F32 = mybir.dt.float32
F32R = mybir.dt.float32r
BF16 = mybir.dt.bfloat16

IBS = 400
I_BLOCKS = [(k * IBS, IBS) for k in range(NI // IBS)]
# jt groups of 2: one exp instruction per group
GROUPS = [tuple(range(g, min(g + 2, JT))) for g in range(0, JT, 2)]


def _build(repeat=1):
    """repeat>1 wraps the whole per-core compute in an on-device For_i loop;
    used only by the perf harness to measure HW time via wall-clock slope."""
    nc = bacc.Bacc(None, target_bir_lowering=False)
    zq = nc.declare_dram_parameter("zq", [CH, NI], F32R, isOutput=False)
    zm = nc.declare_dram_parameter("zm", [128, N], F32R, isOutput=False)
    wqk = nc.declare_dram_parameter("wqk", [CH, 128], F32R, isOutput=False)
    bkq = nc.declare_dram_parameter("bkq", [128, 1], F32, isOutput=False)
    wvT = nc.declare_dram_parameter("wvT", [128, CO], F32R, isOutput=False)
    gbv = nc.declare_dram_parameter("gbv", [CO, 1], F32, isOutput=False)
    gsc = nc.declare_dram_parameter("gsc", [1, 1], F32, isOutput=False)
    ones = nc.declare_dram_parameter("ones", [128, 1], F32R, isOutput=False)
    out = nc.declare_dram_parameter("out", [CO, NI], F32, isOutput=True)

    with tile.TileContext(nc) as tc:
        with (
            tc.tile_pool(name="big", bufs=1) as big,
            tc.tile_pool(name="expp", bufs=6) as expp,
            tc.tile_pool(name="work", bufs=2) as work,
            tc.tile_pool(name="pse", bufs=3, space="PSUM") as pse,
            tc.tile_pool(name="pspv", bufs=1, space="PSUM") as pspv,
        ):
            zm_sb = big.tile([128, N], F32R)
            nc.sync.dma_start(zm_sb[:], zm[:])
            zq_sb = big.tile([CH, NI], F32R)
            nc.sync.dma_start(zq_sb[:], zq[:])
            wqk_sb = big.tile([CH, 128], F32R)
            nc.sync.dma_start(wqk_sb[:], wqk[:])
            bkq_sb = big.tile([128, 1], F32)
            nc.sync.dma_start(bkq_sb[:], bkq[:])
            wv_sb = big.tile([128, CO], F32R)
            nc.sync.dma_start(wv_sb[:], wvT[:])
            gbv_sb = big.tile([CO, 1], F32)
            nc.sync.dma_start(gbv_sb[:], gbv[:])
            gsc_sb = big.tile([1, 1], F32)
            nc.sync.dma_start(gsc_sb[:], gsc[:])
            ones_sb = big.tile([128, 1], F32R)
            nc.sync.dma_start(ones_sb[:], ones[:])

            from contextlib import nullcontext
            rep_ctx = tc.For_i(0, repeat, 1) if repeat > 1 else nullcontext()
            with rep_ctx:
                _emit_body(nc, tc, big, expp, work, pse, pspv,
                           zm_sb, zq_sb, wqk_sb, bkq_sb, wv_sb,
                           gbv_sb, gsc_sb, ones_sb, out)

    nc.finalize()
    return nc


def _emit_body(nc, tc, big, expp, work, pse, pspv,
               zm_sb, zq_sb, wqk_sb, bkq_sb, wv_sb,
               gbv_sb, gsc_sb, ones_sb, out):
    # residual (+ folded gamma*bv), exact fp32 bits of z_hsi
    zqp = big.tile([CH, NI], F32)
    nc.vector.tensor_scalar_add(zqp[:], zq_sb[:].bitcast(F32), gbv_sb[:])

    # QK[c, i] = (Wk^T Wq zq + Wk^T bq)[c, i]   -> E = zm^T QK
    QK_sb = big.tile([128, NI], F32R)
    for c0 in range(0, NI, 512):
        cs = min(512, NI - c0)
        pq = pse.tile([128, 1024], F32, tag="e")
        nc.tensor.matmul(pq[:, :cs], wqk_sb[:], zq_sb[:, c0:c0 + cs],
                         start=True, stop=True)
        nc.vector.tensor_scalar_add(QK_sb[:, c0:c0 + cs], pq[:, :cs],
                                    bkq_sb[:])

    # VT tiles: VT[j, o] = sum_c zm[c, j] Wv[o, c]
    VT_sb = big.tile([128, JT * CO], F32R)
    for g0 in range(0, JT, 4):
        nq = min(4, JT - g0)
        pvt = pse.tile([128, 1024], F32, tag="e")
        for jj in range(nq):
            j0 = (g0 + jj) * 128
            nc.tensor.matmul(pvt[:, jj * 128:(jj + 1) * 128],
                             zm_sb[:, j0:j0 + 128], wv_sb[:],
                             start=True, stop=True)
        nc.vector.tensor_copy(VT_sb[:, g0 * 128:(g0 + nq) * 128],
                              pvt[:, :nq * 128])

    # main attention loop; d/PV matmuls trail one exp-group behind
    for i0, ibs in I_BLOCKS:
        pv = pspv.tile([128, 512], F32, tag="pv")
        dsum = pspv.tile([128, 512], F32, tag="d")

        def emit_dpv(p3_prev, s3_prev, gi):
            grp = GROUPS[gi]
            nc.tensor.matmul(
                dsum[:1, :ibs], ones_sb[:], s3_prev[:, :ibs],
                start=(gi == 0), stop=(gi == len(GROUPS) - 1),
                skip_group_check=True)
            for t, jt in enumerate(grp):
                nc.tensor.matmul(
                    pv[:, :ibs],
                    VT_sb[:, jt * 128:(jt + 1) * 128],
                    p3_prev[:, t * 512:t * 512 + ibs],
                    start=(jt == 0), stop=(jt == JT - 1),
                    skip_group_check=True)

        from collections import deque
        pending = deque()
        for gi, grp in enumerate(GROUPS):
            m = len(grp)
            e3 = pse.tile([128, 1024], F32, tag="e")
            for t, jt in enumerate(grp):
                nc.tensor.matmul(
                    e3[:, t * 512:t * 512 + ibs],
                    zm_sb[:, jt * 128:(jt + 1) * 128],
                    QK_sb[:, i0:i0 + ibs],
                    start=True, stop=True)
            p3 = expp.tile([128, 1024], F32R, tag="p")
            e3v = e3[:].rearrange("p (t x) -> p t x", x=512)[:, :m, :ibs]
            p3v = p3[:].rearrange("p (t x) -> p t x", x=512)[:, :m, :ibs]
            nc.scalar.activation(p3v, e3v, mybir.ActivationFunctionType.Exp)
            s3 = expp.tile([128, 512], F32R, tag="s")
            nc.vector.tensor_add(s3[:, :ibs], p3[:, 0:ibs],
                                 p3[:, 512:512 + ibs])
            pending.append((p3, s3, gi))
            if len(pending) > 3:
                emit_dpv(*pending.popleft())
        while pending:
            emit_dpv(*pending.popleft())

        # normalize: out = PV * (gamma/d) + zqp
        d_inv = work.tile([1, 512], F32, tag="dinv")
        nc.vector.reciprocal(d_inv[:, :ibs], dsum[:1, :ibs])
        d_g = work.tile([1, 512], F32, tag="dg")
        nc.vector.tensor_scalar_mul(d_g[:, :ibs], d_inv[:, :ibs], gsc_sb[:])
        b_sb = work.tile([128, 512], F32, tag="bsb")
        nc.gpsimd.partition_broadcast(b_sb[:, :ibs], d_g[:1, :ibs])
        t_sb = work.tile([128, 512], F32, tag="tsb")
        nc.vector.tensor_mul(t_sb[:, :ibs], pv[:, :ibs], b_sb[:, :ibs])
        o_sb = work.tile([128, 512], F32, tag="osb")
        nc.vector.tensor_add(o_sb[:, :ibs], t_sb[:, :ibs],
                             zqp[:, i0:i0 + ibs])
        nc.sync.dma_start(out[:, i0:i0 + ibs], o_sb[:, :ibs])


_cached_nc = None


def kernel(z_hsi, z_msi, Wq, bq, Wk, bk, Wv, bv, gamma):
    global _cached_nc
    if _cached_nc is None:
        _cached_nc = _build()
    nc = _cached_nc

    z_hsi = np.asarray(z_hsi, dtype=np.float32).reshape(B, CH, N)
    z_msi = np.ascontiguousarray(np.asarray(z_msi, dtype=np.float32).reshape(B, CM, N))
    Wq64 = np.asarray(Wq, dtype=np.float64)
    Wk64 = np.asarray(Wk, dtype=np.float64)
    bq64 = np.asarray(bq, dtype=np.float64)
    # QK folding: E = zm^T (Wk^T Wq zq + Wk^T bq); bk cancels in softmax.
    # All CM=64 contractions are zero-padded to 128: K=64 matmuls run ~2x
    # slower per column on TRN2 than K=128.
    wqk_h = np.zeros((CH, 128), np.float32)
    wqk_h[:, :CM] = (Wq64.T @ Wk64).astype(np.float32)
    bkq_h = np.zeros((128, 1), np.float32)
    bkq_h[:CM, 0] = (Wk64.T @ bq64).astype(np.float32)
    wvT_h = np.zeros((128, CO), np.float32)
    wvT_h[:CM] = np.asarray(Wv, np.float32).T
    z_msi_pad = np.zeros((B, 128, N), np.float32)
    z_msi_pad[:, :CM] = z_msi
    g = float(np.asarray(gamma, dtype=np.float32).reshape(-1)[0])
    gbv = np.ascontiguousarray((g * np.asarray(bv, np.float32)).reshape(CO, 1))
    gsc = np.full((1, 1), g, dtype=np.float32)
    ones = np.ones((128, 1), dtype=np.float32)

    shards_per_b = NCORES // B
    in_maps = []
    for c in range(NCORES):
        b, s = c // shards_per_b, (c % shards_per_b) * NI
        in_maps.append({
            "zq": np.ascontiguousarray(z_hsi[b][:, s:s + NI]),
            "zm": z_msi_pad[b],
            "wqk": wqk_h, "bkq": bkq_h, "wvT": wvT_h,
            "gbv": gbv, "gsc": gsc, "ones": ones,
        })

    res = run_bass_kernel_spmd(nc, in_maps, core_ids=list(range(NCORES)))

    out = np.empty((B, CH, N), dtype=np.float32)
    for c in range(NCORES):
        b, s = c // shards_per_b, (c % shards_per_b) * NI
        out[b][:, s:s + NI] = res.results[c]["out"]
    return out.reshape(B, CH, H, W)
